# revision 31
# baseline (speedup 1.0000x reference)
"""Trainium2 Bass kernel for nn_AttentionBlock_15470472200943.

Causal multi-head attention block (B=8, T=1024, E=1024, H=16, D=64),
data-parallel: one batch element per NeuronCore across 8 cores.

Key transforms vs the straightforward implementation:
- RoPE cancels exactly (module applies the same rotation to q and k at
  every position; R R^T = I inside q k^T) and is skipped.
- Single fused pipeline: v-projection and qk^T-projection matmuls are
  split into ~1.7us half-chain bursts and woven between the attention
  steps of each head pair, so the PE array always has independent work
  while exp results are pending and the exp/normalization work spreads
  across the whole timeline.
- exp splits across engines: ScalarE does exact table exp on/near the
  diagonal; VectorE handles far tiles (queries 512-1023 x keys 0-511
  and the fully-live remainder of r0 tiles) with a one-instruction
  Schraudolph bit trick (int16(23.083*s + 16249) bit-viewed as bf16;
  those entries average over many keys, measured end-to-end rel_l2
  ~8e-3 vs the 2e-2 gate). The 128-wide diagonal strips are masked
  with a tri-mask multiply (VectorE for it0 / GpSimd for it1), and
  attn@v matmuls lag the scores by 6 pipeline steps so exp results
  are ready when consumed.
- Scores stream only causally-live columns (>=256 so fp32r stays at
  1 cycle/row).
- The softmax denominator comes out of the attn@v matmul itself: the
  stationary operand is [ones(64) | v_h(64)] so PSUM rows 0-63 hold
  the sums and rows 64-127 hold y^T.
- Out-projection computes out^T = Wo @ y^T so the output bias becomes
  a per-partition bias folded into the PSUM evacuation; the host
  transposes the result back (free).
"""

import sys

sys.path.insert(0, "/opt/trn_rl_repo")

import math

import ml_dtypes
import numpy as np

import concourse.bass as bass
import concourse.mybir as mybir
import concourse.tile as tile
from concourse import bacc
from concourse.bass_utils import run_bass_kernel_spmd

B, T, E, H = 8, 1024, 1024, 16
D = E // H  # 64
N_CORES = 8
F32 = mybir.dt.float32
F32R = mybir.dt.float32r
BF16 = mybir.dt.bfloat16
I16 = mybir.dt.int16
EXP = mybir.ActivationFunctionType.Exp
IDENT = mybir.ActivationFunctionType.Identity
MULT = mybir.AluOpType.mult
ADD = mybir.AluOpType.add

# Schraudolph constants: exp(0.125*s) ~= bf16_bits(int16(A*s + B16))
A_TRICK = 16.0 / math.log(2.0)     # (128/ln2) * 0.125
B_TRICK = 16256.0 - 7.0

_cache = {}


def _build():
    nc = bacc.Bacc("TRN2", target_bir_lowering=False, debug=False,
                   num_devices=N_CORES)

    # ---- DRAM I/O (per core) ----
    xT = nc.dram_tensor("xT", [T + 1, T], BF16, kind="ExternalInput").ap()
    w_qkT = nc.dram_tensor("w_qkT", [16, 128, 1024], BF16,
                           kind="ExternalInput").ap()
    b_qk = nc.dram_tensor("b_qk", [128, 16], F32, kind="ExternalInput").ap()
    w_vT = nc.dram_tensor("w_vT", [E + 1, E], BF16, kind="ExternalInput").ap()
    w_oT = nc.dram_tensor("w_oT", [8, 128, 1024], BF16,
                          kind="ExternalInput").ap()
    b_o = nc.dram_tensor("b_o", [128, 8], F32, kind="ExternalInput").ap()
    tri = nc.dram_tensor("tri", [128, 2 * 128], BF16, kind="ExternalInput").ap()
    outT = nc.dram_tensor("outT", [E, T], F32, kind="ExternalOutput").ap()

    mm = nc.tensor.matmul

    with tile.TileContext(nc) as tc:
        with (
            tc.tile_pool(name="qkT", bufs=1) as qkT_pool,
            tc.tile_pool(name="v", bufs=1) as v_pool,
            tc.tile_pool(name="misc", bufs=1) as misc_pool,
            tc.tile_pool(name="xTp", bufs=1) as xT_pool,
            tc.tile_pool(name="yTwo", bufs=1) as yTwo_pool,
            tc.tile_pool(name="wqk", bufs=6) as wqk_pool,
        ):
            # ---------- long-lived SBUF ----------
            qkT = qkT_pool.tile([128, 16, 1024], F32R)    # [f%128, f//128, t]
            v_ext = v_pool.tile([128, 8, 16, 128], BF16)  # [k, t, h, ones|v]
            xt = xT_pool.tile([128, 8, 1024], BF16)
            yT = yTwo_pool.tile([128, 8, 1024], BF16)
            wo = yTwo_pool.tile([128, 8, 1024], BF16)
            b_qk_sb = misc_pool.tile([128, 16], F32)
            b_o_sb = misc_pool.tile([128, 8], F32)
            xt_ones = misc_pool.tile([1, 1024], BF16)
            tri_sb = misc_pool.tile([128, 2, 128], BF16)
            scratch = misc_pool.tile([128, 16], F32)

            # ---------- early DMAs ----------
            wqk_tiles = {}

            def wqk_dma(m, ring):
                wqk_tiles[m] = wqk_pool.tile([128, 8, 128], BF16, tag="wqk",
                                             name=f"wqk{m}")
                ring.dma_start(
                    wqk_tiles[m][:].rearrange("p a b -> p (a b)"), w_qkT[m])

            # xt spread over three rings; stationary weights first
            wqk_tiles[0] = wqk_pool.tile([128, 8, 128], BF16, tag="wqk",
                                         name="wqk0")
            nc.sync.dma_start(wqk_tiles[0][:, 0, :], w_qkT[0, :, 0:128])
            nc.sync.dma_start(xt[:, 0, :], xT[0:128, :])
            nc.sync.dma_start(
                wqk_tiles[0][:, 1:8, :].rearrange("p a b -> p (a b)"),
                w_qkT[0, :, 128:1024])
            nc.sync.dma_start(xt[:, 1, :], xT[128:256, :])
            nc.scalar.dma_start(
                xt[:, 4:6, :],
                xT[512:768, :].rearrange("(k p) c -> p k c", p=128))
            nc.scalar.dma_start(
                xt[:, 6:8, :],
                xT[768:1024, :].rearrange("(k p) c -> p k c", p=128))
            nc.sync.dma_start(
                xt[:, 2:4, :],
                xT[256:512, :].rearrange("(k p) c -> p k c", p=128))
            wqk_dma(8, nc.scalar)
            # gpsimd ring: small tensors after the xt chunk
            nc.gpsimd.dma_start(b_qk_sb[:], b_qk[:])
            nc.gpsimd.dma_start(
                tri_sb[:].rearrange("p a b -> p (a b)"), tri[:])
            nc.gpsimd.dma_start(b_o_sb[:], b_o[:])
            nc.gpsimd.dma_start(xt_ones[:], xT[T:T + 1, :])

            # preload the exp table set off the critical path
            nc.scalar.memzero(scratch[:])
            nc.scalar.activation(scratch[:], scratch[:], EXP)

            with (
                tc.tile_pool(name="wv", bufs=1) as wv_pool,
                tc.tile_pool(name="attn", bufs=9) as attn_pool,
                tc.tile_pool(name="rec", bufs=2) as rec_pool,
                tc.tile_pool(name="ps_sc", bufs=2, space="PSUM") as ps_sc,
                tc.tile_pool(name="ps_ys", bufs=2, space="PSUM") as ps_ys,
            ):
                wv = wv_pool.tile([128, 8, 1024], BF16)
                wv_bias = wv_pool.tile([1, 1024], BF16)
                for half in range(2):
                    nc.gpsimd.dma_start(
                        wv[:, 4 * half:4 * half + 4, :],
                        w_vT[512 * half:512 * (half + 1), :].rearrange(
                            "(k p) e -> p k e", p=128))
                nc.gpsimd.dma_start(wv_bias[:], w_vT[E:E + 1, :])
                wqk_dma(1, nc.gpsimd)
                wqk_dma(9, nc.gpsimd)
                # v_ext ones blocks (gpsimd; needed by first attn@v ~25us)
                for t in range(8):
                    nc.gpsimd.memset(v_ext[:, t, :, 0:64], 1.0)

                # ---------- upfront qk^T for head pair 0 ----------------
                # k-major so the chain streams as xt chunks land
                for m in (0, 8):
                    pss = ps_sc.tile([128, 2, 512], F32, tag="sc",
                                     name=f"up{m}")
                    for k in (0, 4, 5, 1, 6, 7, 2, 3):
                        for n in range(2):
                            mm(pss[:, n, :], wqk_tiles[m][:, k, :],
                               xt[:, k, 512 * n:512 * (n + 1)],
                               start=(k == 0), stop=(k == 3))
                    for n in range(2):
                        nc.scalar.activation(
                            qkT[:, m, 512 * n:512 * (n + 1)], pss[:, n, :],
                            IDENT, bias=b_qk_sb[:, m:m + 1])

                # ---------- filler bursts (~1.7us each, 1 PSUM chunk) ---
                v_done = [-1]   # highest fully-issued v t-chunk

                qk_part = {}

                def qk_burst(m, n):
                    qk_burst_a(m, n)
                    qk_burst_b(m, n)

                def qk_burst_a(m, n):
                    pss = ps_sc.tile([128, 2, 512], F32, tag="sc",
                                     name=f"pp{m}_{n}")
                    qk_part[(m, n)] = pss
                    for k in range(4):
                        mm(pss[:, 0, :], wqk_tiles[m][:, k, :],
                           xt[:, k, 512 * n:512 * (n + 1)],
                           start=(k == 0), stop=False)

                def qk_burst_b(m, n):
                    pss = qk_part.pop((m, n))
                    for k in range(4, 8):
                        mm(pss[:, 0, :], wqk_tiles[m][:, k, :],
                           xt[:, k, 512 * n:512 * (n + 1)],
                           start=False, stop=(k == 7))
                    nc.scalar.activation(
                        qkT[:, m, 512 * n:512 * (n + 1)], pss[:, 0, :],
                        IDENT, bias=b_qk_sb[:, m:m + 1])

                def v_burst(t, n):
                    pss = ps_sc.tile([128, 2, 512], F32, tag="sc",
                                     name=f"vv{t}_{n}")
                    for k in range(8):
                        mm(pss[:, 0, :], xt[:, k, 128 * t:128 * (t + 1)],
                           wv[:, k, 512 * n:512 * (n + 1)],
                           start=(k == 0), stop=False)
                    mm(pss[:, 0, :], xt_ones[:, 128 * t:128 * (t + 1)],
                       wv_bias[:, 512 * n:512 * (n + 1)],
                       start=False, stop=True)
                    nc.scalar.copy(
                        v_ext[:, t, 8 * n:8 * (n + 1), 64:128],
                        pss[:, 0, :].rearrange("p (a b) -> p a b", a=8))
                    if n == 1:
                        v_done[0] = t

                bursts = {p: [] for p in range(8)}
                for t in range(8):
                    for n in range(2):
                        bursts[0].append(lambda t=t, n=n: v_burst(t, n))
                for n in range(2):
                    bursts[0].append(lambda n=n: qk_burst(1, n))
                    bursts[0].append(lambda n=n: qk_burst(9, n))
                for p in range(1, 7):
                    for n in range(2):
                        bursts[p].append(lambda m=p + 1, n=n: qk_burst(m, n))
                        bursts[p].append(lambda m=9 + p, n=n: qk_burst(m, n))
                # weight DMA prefetch, one pair ahead (gpsimd ring)
                dma_plan = {p: (p + 2, 10 + p) for p in range(6)}

                jseq = [(0, j) for j in range(4)] + [(1, j) for j in range(8)]
                LAG = 7

                # ---------- attention: one flat pipelined stream --------
                sc_tiles = {}
                at_tiles = {}
                ys_tiles = {}
                steps = [(p, it, jt) for p in range(8) for (it, jt) in jseq]
                # filler bursts keyed to global step index
                pend = []
                sched = {}
                for p in range(8):
                    if p == 0:
                        for i in range(10):
                            sched[12 * p + i] = 2
                    else:
                        for i in (1, 4, 7, 10):
                            sched[12 * p + i] = 1
                flat_bursts = []
                for p in range(8):
                    flat_bursts.append(list(bursts[p]))

                def issue_sc(p, it, jt):
                    r = jt - 4 * it
                    c0 = 0 if r < 0 else min(128 * r, 256)
                    sc = ps_sc.tile([128, 2, 512], F32, tag="sc",
                                    name=f"sc{p}_{it}_{jt}")
                    sc_tiles[(p, it, jt)] = sc
                    for h in range(2):
                        mm(sc[:, h, c0:512],
                           qkT[64 * h:64 * h + 64, 8 + p,
                               128 * jt:128 * (jt + 1)],
                           qkT[64 * h:64 * h + 64, p,
                               512 * it + c0:512 * (it + 1)])

                def issue_exp(p, it, jt):
                    r = jt - 4 * it
                    sc = sc_tiles.pop((p, it, jt))
                    at = attn_pool.tile([128, 2, 512], BF16)
                    at_tiles[(p, it, jt)] = at
                    if r < 0:
                        if jt == 0:
                            # exact exp on ScalarE (balances engines)
                            nc.scalar.activation(at[:], sc[:], EXP,
                                                 scale=0.125)
                        else:
                            nc.vector.tensor_scalar(
                                at[:].bitcast(I16), sc[:],
                                A_TRICK, B_TRICK, MULT, ADD)
                        return
                    lo = 128 * r
                    if r == 0:
                        nc.scalar.activation(at[:, :, 0:128],
                                             sc[:, :, 0:128], EXP,
                                             scale=0.125)
                        nc.vector.tensor_scalar(
                            at[:, :, 128:512].bitcast(I16),
                            sc[:, :, 128:512],
                            A_TRICK, B_TRICK, MULT, ADD)
                    else:
                        nc.scalar.activation(at[:, :, lo:512],
                                             sc[:, :, lo:512], EXP,
                                             scale=0.125)
                    # mask the 128-wide diagonal strip (VectorE for it0,
                    # GpSimd for it1 to balance engine load)
                    eng = nc.vector if it == 0 else nc.gpsimd
                    eng.tensor_mul(at[:, :, lo:lo + 128],
                                   at[:, :, lo:lo + 128], tri_sb[:])

                def issue_av(p, it, jt):
                    r = jt - 4 * it
                    lo = 0 if r < 0 else 128 * r
                    if p == 0:
                        while v_done[0] < jt and flat_bursts[0]:
                            flat_bursts[0].pop(0)()
                    at = at_tiles.pop((p, it, jt))
                    st = (jt == 0)
                    sp = (jt == 3) if it == 0 else (jt == 7)
                    mm(ys_tiles[(p, it)][:, 0, lo:512],
                       v_ext[:, jt, 2 * p, :], at[:, 0, lo:512],
                       start=st, stop=sp)
                    mm(ys_tiles[(p, it)][:, 1, lo:512],
                       v_ext[:, jt, 2 * p + 1, :], at[:, 1, lo:512],
                       start=st, stop=sp)

                deferred = []

                def issue_norm_a(p, it):
                    ys = ys_tiles[(p, it)]
                    rec = rec_pool.tile([64, 2, 512], F32, tag="rec")
                    nc.vector.reciprocal_approx_fast(
                        rec[:], ys[0:64, :, :])
                    deferred.append((p, it, rec))

                def issue_norm_b():
                    p, it, rec = deferred.pop(0)
                    ys = ys_tiles.pop((p, it))
                    nc.vector.tensor_mul(
                        yT[0:64, p, 512 * it:512 * (it + 1)],
                        ys[64:128, 0, :], rec[:, 0, :])
                    nc.vector.tensor_mul(
                        yT[64:128, p, 512 * it:512 * (it + 1)],
                        ys[64:128, 1, :], rec[:, 1, :])

                for g, (p, it, jt) in enumerate(steps):
                    if jt == 0:
                        ys_tiles[(p, it)] = ps_ys.tile(
                            [128, 2, 512], F32, tag="ys", name=f"ys{p}_{it}")
                    if it == 0 and jt == 0:
                        for m in dma_plan.get(p, ()):
                            wqk_dma(m, nc.gpsimd)
                        if p == 4:
                            # prefetch out-projection weights
                            nc.gpsimd.dma_start(
                                wo[:], w_oT.rearrange("a p b -> p a b"))
                    issue_sc(p, it, jt)
                    issue_exp(p, it, jt)
                    if g >= LAG:
                        pp, pit, pjt = steps[g - LAG]
                        issue_av(pp, pit, pjt)
                    if g >= LAG + 2:
                        pp, pit, pjt = steps[g - LAG - 2]
                        if (pit, pjt) == (0, 3) or (pit, pjt) == (1, 7):
                            issue_norm_a(pp, pit)
                    if g >= LAG + 4:
                        pp, pit, pjt = steps[g - LAG - 4]
                        if (pit, pjt) == (0, 3) or (pit, pjt) == (1, 7):
                            issue_norm_b()
                    pb = flat_bursts[p]
                    for _ in range(sched.get(g, 0)):
                        if pb:
                            pb.pop(0)()
                for g in range(len(steps) - LAG, len(steps)):
                    pp, pit, pjt = steps[g]
                    issue_av(pp, pit, pjt)
                done_a = {(p_, it_) for (p_, it_, _) in deferred}
                for (p_, it_) in sorted(ys_tiles.keys()):
                    if (p_, it_) not in done_a:
                        issue_norm_a(p_, it_)
                while deferred:
                    issue_norm_b()

            # ---------------- out^T projection ------------------------
            with (
                tc.tile_pool(name="ost", bufs=4) as out_pool,
                tc.tile_pool(name="ps3", bufs=4, space="PSUM") as ps3,
            ):
                for m2 in range(8):
                    pss = [ps3.tile([128, 512], F32, tag="ps3t",
                                    name=f"o{m2}_{i}") for i in range(2)]
                    for k in range(8):
                        for n in range(2):
                            mm(pss[n][:], wo[:, k, 128 * m2:128 * (m2 + 1)],
                               yT[:, k, 512 * n:512 * (n + 1)],
                               start=(k == 0), stop=(k == 7))
                    for n in range(2):
                        st = out_pool.tile([128, 512], F32)
                        nc.scalar.activation(st[:], pss[n][:], IDENT,
                                             bias=b_o_sb[:, m2:m2 + 1])
                        eng = nc.sync if n == 0 else nc.gpsimd
                        eng.dma_start(
                            outT[128 * m2:128 * (m2 + 1),
                                 512 * n:512 * (n + 1)], st[:])

    nc.compile()
    return nc


def _host_prep(x, w_qkv, b_qkv, w_out, b_out):
    bf = ml_dtypes.bfloat16
    x = np.asarray(x, dtype=np.float32)
    w_qkv = np.asarray(w_qkv, dtype=np.float32)
    b_qkv = np.asarray(b_qkv, dtype=np.float32)
    w_out = np.asarray(w_out, dtype=np.float32)
    b_out = np.asarray(b_out, dtype=np.float32)

    # [m, p, k, c] pre-tiled so each m-tile is one contiguous DMA
    w_qkT = np.ascontiguousarray(
        w_qkv[:2 * E].T.reshape(8, 128, 16, 128).transpose(2, 1, 0, 3)
    ).reshape(16, 128, 1024).astype(bf)
    b_qk = np.ascontiguousarray(
        b_qkv[:2 * E].reshape(16, 128).T).astype(np.float32)     # [128, 16]
    w_vT = np.concatenate(
        [w_qkv[2 * E:].T, b_qkv[2 * E:][None, :]], axis=0).astype(bf)
    # out^T projection stationary tiles:
    # w_oT[k][p][128m+c] = w_out[128m+c, 128k+p]
    w_oT = np.ascontiguousarray(w_out.T.reshape(8, 128, 1024)).astype(bf)
    b_o = np.ascontiguousarray(b_out.reshape(8, 128).T).astype(np.float32)

    j = np.arange(128)[:, None]
    i = np.arange(128)[None, :]
    tri1 = (j <= i).astype(np.float32)
    tri = np.concatenate([tri1, tri1], axis=1).astype(bf)        # [128, 256]

    ones = np.ones((1, T), dtype=np.float32)
    per_core = []
    for cc in range(N_CORES):
        xTc = np.concatenate([x[cc].T, ones], axis=0).astype(bf)
        per_core.append({
            "xT": xTc, "w_qkT": w_qkT, "b_qk": b_qk, "w_vT": w_vT,
            "w_oT": w_oT, "b_o": b_o, "tri": tri,
        })
    return per_core


def kernel(x, w_qkv, b_qkv, w_out, b_out, cos_tab, sin_tab):
    # cos_tab/sin_tab unused: the module applies the identical rotation R to
    # q and k at every position and R R^T = I cancels inside q @ k^T.
    if "nc" not in _cache:
        _cache["nc"] = _build()
    nc = _cache["nc"]
    in_maps = _host_prep(x, w_qkv, b_qkv, w_out, b_out)
    res = run_bass_kernel_spmd(nc, in_maps, list(range(N_CORES)),
                               trace=False)
    out = np.stack([res.results[c]["outT"].T for c in range(N_CORES)], axis=0)
    return np.ascontiguousarray(out).astype(np.float32)


def run_traced(x, w_qkv, b_qkv, w_out, b_out, cos_tab, sin_tab):
    """Like kernel() but with NTFF profiling; returns (out, exec_time_ns,
    trace_path)."""
    if "nc" not in _cache:
        _cache["nc"] = _build()
    nc = _cache["nc"]
    in_maps = _host_prep(x, w_qkv, b_qkv, w_out, b_out)
    res = run_bass_kernel_spmd(nc, in_maps, list(range(N_CORES)), trace=True)
    out = np.stack([res.results[c]["outT"].T for c in range(N_CORES)], axis=0)
    out = np.ascontiguousarray(out).astype(np.float32)
    trace_path = None
    if res.instructions_and_trace is not None:
        trace_path = res.instructions_and_trace[1]
    return out, res.exec_time_ns, trace_path


# revision 32
# speedup vs baseline: 1.0102x; 1.0102x over previous
"""Trainium2 Bass kernel for nn_AttentionBlock_15470472200943.

Causal multi-head attention block (B=8, T=1024, E=1024, H=16, D=64),
data-parallel: one batch element per NeuronCore across 8 cores.

Key transforms vs the straightforward implementation:
- RoPE cancels exactly (module applies the same rotation to q and k at
  every position; R R^T = I inside q k^T) and is skipped.
- Single fused pipeline: v-projection and qk^T-projection matmuls are
  split into ~1.7us half-chain bursts and woven between the attention
  steps of each head pair, so the PE array always has independent work
  while exp results are pending and the exp/normalization work spreads
  across the whole timeline.
- exp splits across engines: ScalarE does exact table exp on/near the
  diagonal; VectorE handles far tiles (queries 512-1023 x keys 0-511
  and the fully-live remainder of r0 tiles) with a one-instruction
  Schraudolph bit trick (int16(23.083*s + 16249) bit-viewed as bf16;
  those entries average over many keys, measured end-to-end rel_l2
  ~8e-3 vs the 2e-2 gate). The 128-wide diagonal strips are masked
  with a tri-mask multiply (VectorE for it0 / GpSimd for it1), and
  attn@v matmuls lag the scores by 6 pipeline steps so exp results
  are ready when consumed.
- Scores stream only causally-live columns (>=256 so fp32r stays at
  1 cycle/row).
- The softmax denominator comes out of the attn@v matmul itself: the
  stationary operand is [ones(64) | v_h(64)] so PSUM rows 0-63 hold
  the sums and rows 64-127 hold y^T.
- Out-projection computes out^T = Wo @ y^T so the output bias becomes
  a per-partition bias folded into the PSUM evacuation; the host
  transposes the result back (free).
"""

import sys

sys.path.insert(0, "/opt/trn_rl_repo")

import math

import ml_dtypes
import numpy as np

import concourse.bass as bass
import concourse.mybir as mybir
import concourse.tile as tile
from concourse import bacc
from concourse.bass_utils import run_bass_kernel_spmd

B, T, E, H = 8, 1024, 1024, 16
D = E // H  # 64
N_CORES = 8
F32 = mybir.dt.float32
F32R = mybir.dt.float32r
BF16 = mybir.dt.bfloat16
I16 = mybir.dt.int16
EXP = mybir.ActivationFunctionType.Exp
IDENT = mybir.ActivationFunctionType.Identity
MULT = mybir.AluOpType.mult
ADD = mybir.AluOpType.add

# Schraudolph constants: exp(0.125*s) ~= bf16_bits(int16(A*s + B16))
A_TRICK = 16.0 / math.log(2.0)     # (128/ln2) * 0.125
B_TRICK = 16256.0 - 7.0

_cache = {}


def _build():
    nc = bacc.Bacc("TRN2", target_bir_lowering=False, debug=False,
                   num_devices=N_CORES)

    # ---- DRAM I/O (per core) ----
    xT = nc.dram_tensor("xT", [T + 1, T], BF16, kind="ExternalInput").ap()
    w_qkT = nc.dram_tensor("w_qkT", [16, 128, 1024], BF16,
                           kind="ExternalInput").ap()
    b_qk = nc.dram_tensor("b_qk", [128, 16], F32, kind="ExternalInput").ap()
    w_vT = nc.dram_tensor("w_vT", [E + 1, E], BF16, kind="ExternalInput").ap()
    w_oT = nc.dram_tensor("w_oT", [8, 128, 1024], BF16,
                          kind="ExternalInput").ap()
    b_o = nc.dram_tensor("b_o", [128, 8], F32, kind="ExternalInput").ap()
    tri = nc.dram_tensor("tri", [128, 2 * 128], BF16, kind="ExternalInput").ap()
    outT = nc.dram_tensor("outT", [E, T], F32, kind="ExternalOutput").ap()

    mm = nc.tensor.matmul

    with tile.TileContext(nc) as tc:
        with (
            tc.tile_pool(name="qkT", bufs=1) as qkT_pool,
            tc.tile_pool(name="v", bufs=1) as v_pool,
            tc.tile_pool(name="misc", bufs=1) as misc_pool,
            tc.tile_pool(name="xTp", bufs=1) as xT_pool,
            tc.tile_pool(name="yTwo", bufs=1) as yTwo_pool,
            tc.tile_pool(name="wqk", bufs=6) as wqk_pool,
        ):
            # ---------- long-lived SBUF ----------
            qkT = qkT_pool.tile([128, 16, 1024], F32R)    # [f%128, f//128, t]
            v_ext = v_pool.tile([128, 8, 16, 128], BF16)  # [k, t, h, ones|v]
            xt = xT_pool.tile([128, 8, 1024], BF16)
            yT = yTwo_pool.tile([128, 8, 1024], BF16)
            wo = yTwo_pool.tile([128, 8, 1024], BF16)
            b_qk_sb = misc_pool.tile([128, 16], F32)
            b_o_sb = misc_pool.tile([128, 8], F32)
            xt_ones = misc_pool.tile([1, 1024], BF16)
            tri_sb = misc_pool.tile([128, 2, 128], BF16)
            scratch = misc_pool.tile([128, 16], F32)

            # ---------- early DMAs ----------
            wqk_tiles = {}

            def wqk_dma(m, ring):
                wqk_tiles[m] = wqk_pool.tile([128, 8, 128], BF16, tag="wqk",
                                             name=f"wqk{m}")
                ring.dma_start(
                    wqk_tiles[m][:].rearrange("p a b -> p (a b)"), w_qkT[m])

            # xt spread over three rings; stationary weights first
            wqk_tiles[0] = wqk_pool.tile([128, 8, 128], BF16, tag="wqk",
                                         name="wqk0")
            nc.sync.dma_start(wqk_tiles[0][:, 0, :], w_qkT[0, :, 0:128])
            nc.sync.dma_start(xt[:, 0, :], xT[0:128, :])
            nc.sync.dma_start(
                wqk_tiles[0][:, 1:8, :].rearrange("p a b -> p (a b)"),
                w_qkT[0, :, 128:1024])
            nc.sync.dma_start(xt[:, 1, :], xT[128:256, :])
            nc.scalar.dma_start(
                xt[:, 4:6, :],
                xT[512:768, :].rearrange("(k p) c -> p k c", p=128))
            nc.scalar.dma_start(
                xt[:, 6:8, :],
                xT[768:1024, :].rearrange("(k p) c -> p k c", p=128))
            nc.sync.dma_start(
                xt[:, 2:4, :],
                xT[256:512, :].rearrange("(k p) c -> p k c", p=128))
            wqk_dma(8, nc.scalar)
            # gpsimd ring: small tensors after the xt chunk
            nc.gpsimd.dma_start(b_qk_sb[:], b_qk[:])
            nc.gpsimd.dma_start(
                tri_sb[:].rearrange("p a b -> p (a b)"), tri[:])
            nc.gpsimd.dma_start(b_o_sb[:], b_o[:])
            nc.gpsimd.dma_start(xt_ones[:], xT[T:T + 1, :])

            # preload the exp table set off the critical path
            nc.scalar.memzero(scratch[:])
            nc.scalar.activation(scratch[:], scratch[:], EXP)

            with (
                tc.tile_pool(name="wv", bufs=1) as wv_pool,
                tc.tile_pool(name="attn", bufs=7) as attn_pool,
                tc.tile_pool(name="rec", bufs=2) as rec_pool,
                tc.tile_pool(name="ps_sc", bufs=2, space="PSUM") as ps_sc,
                tc.tile_pool(name="ps_ys", bufs=2, space="PSUM") as ps_ys,
            ):
                wv = wv_pool.tile([128, 8, 1024], BF16)
                wv_bias = wv_pool.tile([1, 1024], BF16)
                for half in range(2):
                    nc.gpsimd.dma_start(
                        wv[:, 4 * half:4 * half + 4, :],
                        w_vT[512 * half:512 * (half + 1), :].rearrange(
                            "(k p) e -> p k e", p=128))
                nc.gpsimd.dma_start(wv_bias[:], w_vT[E:E + 1, :])
                wqk_dma(1, nc.gpsimd)
                wqk_dma(9, nc.gpsimd)
                # v_ext ones blocks (gpsimd; needed by first attn@v ~25us)
                for t in range(8):
                    nc.gpsimd.memset(v_ext[:, t, :, 0:64], 1.0)

                # ---------- upfront qk^T for head pair 0 ----------------
                # k-major so the chain streams as xt chunks land
                for m in (0, 8):
                    pss = ps_sc.tile([128, 2, 512], F32, tag="sc",
                                     name=f"up{m}")
                    for k in (0, 4, 5, 1, 6, 7, 2, 3):
                        for n in range(2):
                            mm(pss[:, n, :], wqk_tiles[m][:, k, :],
                               xt[:, k, 512 * n:512 * (n + 1)],
                               start=(k == 0), stop=(k == 3))
                    for n in range(2):
                        nc.scalar.activation(
                            qkT[:, m, 512 * n:512 * (n + 1)], pss[:, n, :],
                            IDENT, bias=b_qk_sb[:, m:m + 1])

                # ---------- filler bursts (~1.7us each, 1 PSUM chunk) ---
                v_done = [-1]   # highest fully-issued v t-chunk

                qk_part = {}

                def qk_burst(m, n):
                    qk_burst_a(m, n)
                    qk_burst_b(m, n)

                def qk_burst_a(m, n):
                    pss = ps_sc.tile([128, 2, 512], F32, tag="sc",
                                     name=f"pp{m}_{n}")
                    qk_part[(m, n)] = pss
                    for k in range(4):
                        mm(pss[:, 0, :], wqk_tiles[m][:, k, :],
                           xt[:, k, 512 * n:512 * (n + 1)],
                           start=(k == 0), stop=False)

                def qk_burst_b(m, n):
                    pss = qk_part.pop((m, n))
                    for k in range(4, 8):
                        mm(pss[:, 0, :], wqk_tiles[m][:, k, :],
                           xt[:, k, 512 * n:512 * (n + 1)],
                           start=False, stop=(k == 7))
                    nc.scalar.activation(
                        qkT[:, m, 512 * n:512 * (n + 1)], pss[:, 0, :],
                        IDENT, bias=b_qk_sb[:, m:m + 1])

                def v_burst(t, n):
                    pss = ps_sc.tile([128, 2, 512], F32, tag="sc",
                                     name=f"vv{t}_{n}")
                    for k in range(8):
                        mm(pss[:, 0, :], xt[:, k, 128 * t:128 * (t + 1)],
                           wv[:, k, 512 * n:512 * (n + 1)],
                           start=(k == 0), stop=False)
                    mm(pss[:, 0, :], xt_ones[:, 128 * t:128 * (t + 1)],
                       wv_bias[:, 512 * n:512 * (n + 1)],
                       start=False, stop=True)
                    nc.scalar.copy(
                        v_ext[:, t, 8 * n:8 * (n + 1), 64:128],
                        pss[:, 0, :].rearrange("p (a b) -> p a b", a=8))
                    if n == 1:
                        v_done[0] = t

                bursts = {p: [] for p in range(8)}
                for t in range(8):
                    for n in range(2):
                        bursts[0].append(lambda t=t, n=n: v_burst(t, n))
                for n in range(2):
                    bursts[0].append(lambda n=n: qk_burst(1, n))
                    bursts[0].append(lambda n=n: qk_burst(9, n))
                for p in range(1, 7):
                    for n in range(2):
                        bursts[p].append(lambda m=p + 1, n=n: qk_burst(m, n))
                        bursts[p].append(lambda m=9 + p, n=n: qk_burst(m, n))
                # weight DMA prefetch, one pair ahead (gpsimd ring)
                dma_plan = {p: (p + 2, 10 + p) for p in range(6)}

                jseq = [(0, j) for j in range(4)] + [(1, j) for j in range(8)]
                LAG = 6

                # ---------- attention: one flat pipelined stream --------
                sc_tiles = {}
                at_tiles = {}
                ys_tiles = {}
                steps = [(p, it, jt) for p in range(8) for (it, jt) in jseq]
                # filler bursts keyed to global step index
                pend = []
                sched = {}
                for p in range(8):
                    if p == 0:
                        for i in range(10):
                            sched[12 * p + i] = 2
                    else:
                        for i in (1, 4, 7, 10):
                            sched[12 * p + i] = 1
                flat_bursts = []
                for p in range(8):
                    flat_bursts.append(list(bursts[p]))

                def issue_sc(p, it, jt):
                    r = jt - 4 * it
                    c0 = 0 if r < 0 else min(128 * r, 256)
                    sc = ps_sc.tile([128, 2, 512], F32, tag="sc",
                                    name=f"sc{p}_{it}_{jt}")
                    sc_tiles[(p, it, jt)] = sc
                    for h in range(2):
                        mm(sc[:, h, c0:512],
                           qkT[64 * h:64 * h + 64, 8 + p,
                               128 * jt:128 * (jt + 1)],
                           qkT[64 * h:64 * h + 64, p,
                               512 * it + c0:512 * (it + 1)])

                def issue_exp(p, it, jt):
                    r = jt - 4 * it
                    sc = sc_tiles.pop((p, it, jt))
                    at = attn_pool.tile([128, 2, 512], BF16)
                    at_tiles[(p, it, jt)] = at
                    if r < 0:
                        if jt == 0:
                            # exact exp on ScalarE (balances engines)
                            nc.scalar.activation(at[:], sc[:], EXP,
                                                 scale=0.125)
                        else:
                            nc.vector.tensor_scalar(
                                at[:].bitcast(I16), sc[:],
                                A_TRICK, B_TRICK, MULT, ADD)
                        return
                    lo = 128 * r
                    if r == 0:
                        nc.scalar.activation(at[:, :, 0:128],
                                             sc[:, :, 0:128], EXP,
                                             scale=0.125)
                        nc.vector.tensor_scalar(
                            at[:, :, 128:512].bitcast(I16),
                            sc[:, :, 128:512],
                            A_TRICK, B_TRICK, MULT, ADD)
                    else:
                        nc.scalar.activation(at[:, :, lo:512],
                                             sc[:, :, lo:512], EXP,
                                             scale=0.125)
                    # mask the 128-wide diagonal strip (VectorE for it0,
                    # GpSimd for it1 to balance engine load)
                    eng = nc.vector if it == 0 else nc.gpsimd
                    eng.tensor_mul(at[:, :, lo:lo + 128],
                                   at[:, :, lo:lo + 128], tri_sb[:])

                def issue_av(p, it, jt):
                    r = jt - 4 * it
                    lo = 0 if r < 0 else 128 * r
                    if p == 0:
                        while v_done[0] < jt and flat_bursts[0]:
                            flat_bursts[0].pop(0)()
                    at = at_tiles.pop((p, it, jt))
                    st = (jt == 0)
                    sp = (jt == 3) if it == 0 else (jt == 7)
                    mm(ys_tiles[(p, it)][:, 0, lo:512],
                       v_ext[:, jt, 2 * p, :], at[:, 0, lo:512],
                       start=st, stop=sp)
                    mm(ys_tiles[(p, it)][:, 1, lo:512],
                       v_ext[:, jt, 2 * p + 1, :], at[:, 1, lo:512],
                       start=st, stop=sp)

                deferred = []

                def issue_norm_a(p, it):
                    ys = ys_tiles[(p, it)]
                    rec = rec_pool.tile([64, 2, 512], F32, tag="rec")
                    nc.vector.reciprocal_approx_fast(
                        rec[:], ys[0:64, :, :])
                    deferred.append((p, it, rec))

                def issue_norm_b():
                    p, it, rec = deferred.pop(0)
                    ys = ys_tiles.pop((p, it))
                    nc.vector.tensor_mul(
                        yT[0:64, p, 512 * it:512 * (it + 1)],
                        ys[64:128, 0, :], rec[:, 0, :])
                    nc.vector.tensor_mul(
                        yT[64:128, p, 512 * it:512 * (it + 1)],
                        ys[64:128, 1, :], rec[:, 1, :])

                for g, (p, it, jt) in enumerate(steps):
                    if jt == 0:
                        ys_tiles[(p, it)] = ps_ys.tile(
                            [128, 2, 512], F32, tag="ys", name=f"ys{p}_{it}")
                    if it == 0 and jt == 0:
                        for m in dma_plan.get(p, ()):
                            wqk_dma(m, nc.gpsimd)
                        if p == 4:
                            # prefetch out-projection weights
                            nc.gpsimd.dma_start(
                                wo[:], w_oT.rearrange("a p b -> p a b"))
                    issue_sc(p, it, jt)
                    issue_exp(p, it, jt)
                    if g >= LAG:
                        pp, pit, pjt = steps[g - LAG]
                        issue_av(pp, pit, pjt)
                        if (pit, pjt) == (0, 3) or (pit, pjt) == (1, 7):
                            issue_norm_a(pp, pit)
                    if g >= LAG + 2:
                        pp, pit, pjt = steps[g - LAG - 2]
                        if (pit, pjt) == (0, 3) or (pit, pjt) == (1, 7):
                            issue_norm_b()
                    pb = flat_bursts[p]
                    for _ in range(sched.get(g, 0)):
                        if pb:
                            pb.pop(0)()
                for g in range(len(steps) - LAG, len(steps)):
                    pp, pit, pjt = steps[g]
                    issue_av(pp, pit, pjt)
                    if (pit, pjt) == (0, 3) or (pit, pjt) == (1, 7):
                        issue_norm_a(pp, pit)
                while deferred:
                    issue_norm_b()

            # ---------------- out^T projection ------------------------
            with (
                tc.tile_pool(name="ost", bufs=4) as out_pool,
                tc.tile_pool(name="ps3", bufs=4, space="PSUM") as ps3,
            ):
                for m2 in range(8):
                    pss = [ps3.tile([128, 512], F32, tag="ps3t",
                                    name=f"o{m2}_{i}") for i in range(2)]
                    for k in range(8):
                        for n in range(2):
                            mm(pss[n][:], wo[:, k, 128 * m2:128 * (m2 + 1)],
                               yT[:, k, 512 * n:512 * (n + 1)],
                               start=(k == 0), stop=(k == 7))
                    for n in range(2):
                        st = out_pool.tile([128, 512], F32)
                        nc.scalar.activation(st[:], pss[n][:], IDENT,
                                             bias=b_o_sb[:, m2:m2 + 1])
                        eng = nc.sync if n == 0 else nc.gpsimd
                        eng.dma_start(
                            outT[128 * m2:128 * (m2 + 1),
                                 512 * n:512 * (n + 1)], st[:])

    nc.compile()
    return nc


def _host_prep(x, w_qkv, b_qkv, w_out, b_out):
    bf = ml_dtypes.bfloat16
    x = np.asarray(x, dtype=np.float32)
    w_qkv = np.asarray(w_qkv, dtype=np.float32)
    b_qkv = np.asarray(b_qkv, dtype=np.float32)
    w_out = np.asarray(w_out, dtype=np.float32)
    b_out = np.asarray(b_out, dtype=np.float32)

    # [m, p, k, c] pre-tiled so each m-tile is one contiguous DMA
    w_qkT = np.ascontiguousarray(
        w_qkv[:2 * E].T.reshape(8, 128, 16, 128).transpose(2, 1, 0, 3)
    ).reshape(16, 128, 1024).astype(bf)
    b_qk = np.ascontiguousarray(
        b_qkv[:2 * E].reshape(16, 128).T).astype(np.float32)     # [128, 16]
    w_vT = np.concatenate(
        [w_qkv[2 * E:].T, b_qkv[2 * E:][None, :]], axis=0).astype(bf)
    # out^T projection stationary tiles:
    # w_oT[k][p][128m+c] = w_out[128m+c, 128k+p]
    w_oT = np.ascontiguousarray(w_out.T.reshape(8, 128, 1024)).astype(bf)
    b_o = np.ascontiguousarray(b_out.reshape(8, 128).T).astype(np.float32)

    j = np.arange(128)[:, None]
    i = np.arange(128)[None, :]
    tri1 = (j <= i).astype(np.float32)
    tri = np.concatenate([tri1, tri1], axis=1).astype(bf)        # [128, 256]

    ones = np.ones((1, T), dtype=np.float32)
    per_core = []
    for cc in range(N_CORES):
        xTc = np.concatenate([x[cc].T, ones], axis=0).astype(bf)
        per_core.append({
            "xT": xTc, "w_qkT": w_qkT, "b_qk": b_qk, "w_vT": w_vT,
            "w_oT": w_oT, "b_o": b_o, "tri": tri,
        })
    return per_core


def kernel(x, w_qkv, b_qkv, w_out, b_out, cos_tab, sin_tab):
    # cos_tab/sin_tab unused: the module applies the identical rotation R to
    # q and k at every position and R R^T = I cancels inside q @ k^T.
    if "nc" not in _cache:
        _cache["nc"] = _build()
    nc = _cache["nc"]
    in_maps = _host_prep(x, w_qkv, b_qkv, w_out, b_out)
    res = run_bass_kernel_spmd(nc, in_maps, list(range(N_CORES)),
                               trace=False)
    out = np.stack([res.results[c]["outT"].T for c in range(N_CORES)], axis=0)
    return np.ascontiguousarray(out).astype(np.float32)


def run_traced(x, w_qkv, b_qkv, w_out, b_out, cos_tab, sin_tab):
    """Like kernel() but with NTFF profiling; returns (out, exec_time_ns,
    trace_path)."""
    if "nc" not in _cache:
        _cache["nc"] = _build()
    nc = _cache["nc"]
    in_maps = _host_prep(x, w_qkv, b_qkv, w_out, b_out)
    res = run_bass_kernel_spmd(nc, in_maps, list(range(N_CORES)), trace=True)
    out = np.stack([res.results[c]["outT"].T for c in range(N_CORES)], axis=0)
    out = np.ascontiguousarray(out).astype(np.float32)
    trace_path = None
    if res.instructions_and_trace is not None:
        trace_path = res.instructions_and_trace[1]
    return out, res.exec_time_ns, trace_path


# revision 33
# speedup vs baseline: 1.0138x; 1.0036x over previous
"""Trainium2 Bass kernel for nn_AttentionBlock_15470472200943.

Causal multi-head attention block (B=8, T=1024, E=1024, H=16, D=64),
data-parallel: one batch element per NeuronCore across 8 cores.

Key transforms vs the straightforward implementation:
- RoPE cancels exactly (module applies the same rotation to q and k at
  every position; R R^T = I inside q k^T) and is skipped.
- Single fused pipeline: v-projection and qk^T-projection matmuls are
  split into ~1.7us half-chain bursts and woven between the attention
  steps of each head pair, so the PE array always has independent work
  while exp results are pending and the exp/normalization work spreads
  across the whole timeline.
- exp splits across engines: ScalarE does exact table exp on/near the
  diagonal; VectorE handles far tiles (queries 512-1023 x keys 0-511
  and the fully-live remainder of r0 tiles) with a one-instruction
  Schraudolph bit trick (int16(23.083*s + 16249) bit-viewed as bf16;
  those entries average over many keys, measured end-to-end rel_l2
  ~8e-3 vs the 2e-2 gate). The 128-wide diagonal strips are masked
  with a tri-mask multiply (VectorE for it0 / GpSimd for it1), and
  attn@v matmuls lag the scores by 6 pipeline steps so exp results
  are ready when consumed.
- Scores stream only causally-live columns (>=256 so fp32r stays at
  1 cycle/row).
- The softmax denominator comes out of the attn@v matmul itself: the
  stationary operand is [ones(64) | v_h(64)] so PSUM rows 0-63 hold
  the sums and rows 64-127 hold y^T.
- Out-projection computes out^T = Wo @ y^T so the output bias becomes
  a per-partition bias folded into the PSUM evacuation; the host
  transposes the result back (free).
"""

import sys

sys.path.insert(0, "/opt/trn_rl_repo")

import math

import ml_dtypes
import numpy as np

import concourse.bass as bass
import concourse.mybir as mybir
import concourse.tile as tile
from concourse import bacc
from concourse.bass_utils import run_bass_kernel_spmd

B, T, E, H = 8, 1024, 1024, 16
D = E // H  # 64
N_CORES = 8
F32 = mybir.dt.float32
F32R = mybir.dt.float32r
BF16 = mybir.dt.bfloat16
I16 = mybir.dt.int16
EXP = mybir.ActivationFunctionType.Exp
IDENT = mybir.ActivationFunctionType.Identity
MULT = mybir.AluOpType.mult
ADD = mybir.AluOpType.add

# Schraudolph constants: exp(0.125*s) ~= bf16_bits(int16(A*s + B16))
A_TRICK = 16.0 / math.log(2.0)     # (128/ln2) * 0.125
B_TRICK = 16256.0 - 7.0

_cache = {}


def _build():
    nc = bacc.Bacc("TRN2", target_bir_lowering=False, debug=False,
                   num_devices=N_CORES)

    # ---- DRAM I/O (per core) ----
    xT = nc.dram_tensor("xT", [T + 1, T], BF16, kind="ExternalInput").ap()
    w_qkT = nc.dram_tensor("w_qkT", [16, 128, 1024], BF16,
                           kind="ExternalInput").ap()
    b_qk = nc.dram_tensor("b_qk", [128, 16], F32, kind="ExternalInput").ap()
    w_vT = nc.dram_tensor("w_vT", [E + 1, E], BF16, kind="ExternalInput").ap()
    w_oT = nc.dram_tensor("w_oT", [8, 128, 1024], BF16,
                          kind="ExternalInput").ap()
    b_o = nc.dram_tensor("b_o", [128, 8], F32, kind="ExternalInput").ap()
    tri = nc.dram_tensor("tri", [128, 2 * 128], BF16, kind="ExternalInput").ap()
    outT = nc.dram_tensor("outT", [E, T], F32, kind="ExternalOutput").ap()

    mm = nc.tensor.matmul

    with tile.TileContext(nc) as tc:
        with (
            tc.tile_pool(name="qkT", bufs=1) as qkT_pool,
            tc.tile_pool(name="v", bufs=1) as v_pool,
            tc.tile_pool(name="misc", bufs=1) as misc_pool,
            tc.tile_pool(name="xTp", bufs=1) as xT_pool,
            tc.tile_pool(name="yTwo", bufs=1) as yTwo_pool,
            tc.tile_pool(name="wqk", bufs=6) as wqk_pool,
        ):
            # ---------- long-lived SBUF ----------
            qkT = qkT_pool.tile([128, 16, 1024], F32R)    # [f%128, f//128, t]
            v_ext = v_pool.tile([128, 8, 16, 128], BF16)  # [k, t, h, ones|v]
            xt = xT_pool.tile([128, 8, 1024], BF16)
            yT = yTwo_pool.tile([128, 8, 1024], BF16)
            wo = yTwo_pool.tile([128, 8, 1024], BF16)
            b_qk_sb = misc_pool.tile([128, 16], F32)
            b_o_sb = misc_pool.tile([128, 8], F32)
            xt_ones = misc_pool.tile([1, 1024], BF16)
            tri_sb = misc_pool.tile([128, 2, 128], BF16)
            scratch = misc_pool.tile([128, 16], F32)

            # ---------- early DMAs ----------
            wqk_tiles = {}

            def wqk_dma(m, ring):
                wqk_tiles[m] = wqk_pool.tile([128, 8, 128], BF16, tag="wqk",
                                             name=f"wqk{m}")
                ring.dma_start(
                    wqk_tiles[m][:].rearrange("p a b -> p (a b)"), w_qkT[m])

            # xt spread over three rings; stationary weights first
            wqk_tiles[0] = wqk_pool.tile([128, 8, 128], BF16, tag="wqk",
                                         name="wqk0")
            nc.sync.dma_start(wqk_tiles[0][:, 0, :], w_qkT[0, :, 0:128])
            nc.sync.dma_start(xt[:, 0, :], xT[0:128, :])
            nc.sync.dma_start(
                wqk_tiles[0][:, 1:8, :].rearrange("p a b -> p (a b)"),
                w_qkT[0, :, 128:1024])
            nc.sync.dma_start(xt[:, 1, :], xT[128:256, :])
            nc.scalar.dma_start(
                xt[:, 4:6, :],
                xT[512:768, :].rearrange("(k p) c -> p k c", p=128))
            nc.scalar.dma_start(
                xt[:, 6:8, :],
                xT[768:1024, :].rearrange("(k p) c -> p k c", p=128))
            nc.sync.dma_start(
                xt[:, 2:4, :],
                xT[256:512, :].rearrange("(k p) c -> p k c", p=128))
            wqk_dma(8, nc.scalar)
            # gpsimd ring: small tensors after the xt chunk
            nc.gpsimd.dma_start(b_qk_sb[:], b_qk[:])
            nc.gpsimd.dma_start(
                tri_sb[:].rearrange("p a b -> p (a b)"), tri[:])
            nc.gpsimd.dma_start(b_o_sb[:], b_o[:])
            nc.gpsimd.dma_start(xt_ones[:], xT[T:T + 1, :])

            # preload the exp table set off the critical path
            nc.scalar.memzero(scratch[:])
            nc.scalar.activation(scratch[:], scratch[:], EXP)

            with (
                tc.tile_pool(name="wv", bufs=1) as wv_pool,
                tc.tile_pool(name="attn", bufs=7) as attn_pool,
                tc.tile_pool(name="rec", bufs=2) as rec_pool,
                tc.tile_pool(name="ps_sc", bufs=2, space="PSUM") as ps_sc,
                tc.tile_pool(name="ps_ys", bufs=2, space="PSUM") as ps_ys,
            ):
                wv = wv_pool.tile([128, 8, 1024], BF16)
                wv_bias = wv_pool.tile([1, 1024], BF16)
                for half in range(2):
                    nc.gpsimd.dma_start(
                        wv[:, 4 * half:4 * half + 4, :],
                        w_vT[512 * half:512 * (half + 1), :].rearrange(
                            "(k p) e -> p k e", p=128))
                nc.gpsimd.dma_start(wv_bias[:], w_vT[E:E + 1, :])
                wqk_dma(1, nc.gpsimd)
                wqk_dma(9, nc.gpsimd)
                # v_ext ones blocks (gpsimd; needed by first attn@v ~25us)
                for t in range(8):
                    nc.gpsimd.memset(v_ext[:, t, :, 0:64], 1.0)

                # ---------- upfront qk^T for head pair 0 ----------------
                # k-major so the chain streams as xt chunks land
                for m in (0, 8):
                    pss = ps_sc.tile([128, 2, 512], F32, tag="sc",
                                     name=f"up{m}")
                    for k in (0, 4, 5, 1, 6, 7, 2, 3):
                        for n in range(2):
                            mm(pss[:, n, :], wqk_tiles[m][:, k, :],
                               xt[:, k, 512 * n:512 * (n + 1)],
                               start=(k == 0), stop=(k == 3))
                    for n in range(2):
                        nc.scalar.activation(
                            qkT[:, m, 512 * n:512 * (n + 1)], pss[:, n, :],
                            IDENT, bias=b_qk_sb[:, m:m + 1])

                # ---------- filler bursts (~1.7us each, 1 PSUM chunk) ---
                v_done = [-1]   # highest fully-issued v t-chunk

                qk_part = {}

                def qk_burst(m, n):
                    qk_burst_a(m, n)
                    qk_burst_b(m, n)

                def qk_burst_a(m, n):
                    pss = ps_sc.tile([128, 2, 512], F32, tag="sc",
                                     name=f"pp{m}_{n}")
                    qk_part[(m, n)] = pss
                    for k in range(4):
                        mm(pss[:, 0, :], wqk_tiles[m][:, k, :],
                           xt[:, k, 512 * n:512 * (n + 1)],
                           start=(k == 0), stop=False)

                def qk_burst_b(m, n):
                    pss = qk_part.pop((m, n))
                    for k in range(4, 8):
                        mm(pss[:, 0, :], wqk_tiles[m][:, k, :],
                           xt[:, k, 512 * n:512 * (n + 1)],
                           start=False, stop=(k == 7))
                    nc.scalar.activation(
                        qkT[:, m, 512 * n:512 * (n + 1)], pss[:, 0, :],
                        IDENT, bias=b_qk_sb[:, m:m + 1])

                def v_burst(t, n):
                    pss = ps_sc.tile([128, 2, 512], F32, tag="sc",
                                     name=f"vv{t}_{n}")
                    for k in range(8):
                        mm(pss[:, 0, :], xt[:, k, 128 * t:128 * (t + 1)],
                           wv[:, k, 512 * n:512 * (n + 1)],
                           start=(k == 0), stop=False)
                    mm(pss[:, 0, :], xt_ones[:, 128 * t:128 * (t + 1)],
                       wv_bias[:, 512 * n:512 * (n + 1)],
                       start=False, stop=True)
                    nc.scalar.copy(
                        v_ext[:, t, 8 * n:8 * (n + 1), 64:128],
                        pss[:, 0, :].rearrange("p (a b) -> p a b", a=8))
                    if n == 1:
                        v_done[0] = t

                bursts = {p: [] for p in range(8)}
                for t in range(8):
                    for n in range(2):
                        bursts[0].append(lambda t=t, n=n: v_burst(t, n))
                for n in range(2):
                    bursts[0].append(lambda n=n: qk_burst(1, n))
                    bursts[0].append(lambda n=n: qk_burst(9, n))
                for p in range(1, 7):
                    for n in range(2):
                        bursts[p].append(lambda m=p + 1, n=n: qk_burst(m, n))
                        bursts[p].append(lambda m=9 + p, n=n: qk_burst(m, n))
                # weight DMA prefetch, one pair ahead (gpsimd ring)
                dma_plan = {p: (p + 2, 10 + p) for p in range(6)}

                jseq = [(0, j) for j in range(4)] + [(1, j) for j in range(8)]
                LAG = 6

                # ---------- attention: one flat pipelined stream --------
                sc_tiles = {}
                at_tiles = {}
                ys_tiles = {}
                steps = [(p, it, jt) for p in range(8) for (it, jt) in jseq]
                # filler bursts keyed to global step index
                pend = []
                sched = {}
                for p in range(8):
                    if p == 0:
                        for i in range(10):
                            sched[12 * p + i] = 2
                    else:
                        for i in (1, 4, 7, 10):
                            sched[12 * p + i] = 1
                flat_bursts = []
                for p in range(8):
                    flat_bursts.append(list(bursts[p]))

                def issue_sc(p, it, jt):
                    r = jt - 4 * it
                    c0 = 0 if r < 0 else min(128 * r, 256)
                    sc = ps_sc.tile([128, 2, 512], F32, tag="sc",
                                    name=f"sc{p}_{it}_{jt}")
                    sc_tiles[(p, it, jt)] = sc
                    for h in range(2):
                        mm(sc[:, h, c0:512],
                           qkT[64 * h:64 * h + 64, 8 + p,
                               128 * jt:128 * (jt + 1)],
                           qkT[64 * h:64 * h + 64, p,
                               512 * it + c0:512 * (it + 1)])

                def issue_exp(p, it, jt):
                    r = jt - 4 * it
                    sc = sc_tiles.pop((p, it, jt))
                    at = attn_pool.tile([128, 2, 512], BF16)
                    at_tiles[(p, it, jt)] = at
                    if r < 0:
                        if jt == 0:
                            # exact exp on ScalarE (balances engines)
                            nc.scalar.activation(at[:], sc[:], EXP,
                                                 scale=0.125)
                        else:
                            nc.vector.tensor_scalar(
                                at[:].bitcast(I16), sc[:],
                                A_TRICK, B_TRICK, MULT, ADD)
                        return
                    lo = 128 * r
                    if r == 0:
                        nc.scalar.activation(at[:, :, 0:128],
                                             sc[:, :, 0:128], EXP,
                                             scale=0.125)
                        nc.vector.tensor_scalar(
                            at[:, :, 128:512].bitcast(I16),
                            sc[:, :, 128:512],
                            A_TRICK, B_TRICK, MULT, ADD)
                    else:
                        nc.scalar.activation(at[:, :, lo:512],
                                             sc[:, :, lo:512], EXP,
                                             scale=0.125)
                    # mask the 128-wide diagonal strip on GpSimd (keeps
                    # VectorE free for trick exps; LAG covers the latency)
                    nc.gpsimd.tensor_mul(at[:, :, lo:lo + 128],
                                         at[:, :, lo:lo + 128], tri_sb[:])

                def issue_av(p, it, jt):
                    r = jt - 4 * it
                    lo = 0 if r < 0 else 128 * r
                    if p == 0:
                        while v_done[0] < jt and flat_bursts[0]:
                            flat_bursts[0].pop(0)()
                    at = at_tiles.pop((p, it, jt))
                    st = (jt == 0)
                    sp = (jt == 3) if it == 0 else (jt == 7)
                    mm(ys_tiles[(p, it)][:, 0, lo:512],
                       v_ext[:, jt, 2 * p, :], at[:, 0, lo:512],
                       start=st, stop=sp)
                    mm(ys_tiles[(p, it)][:, 1, lo:512],
                       v_ext[:, jt, 2 * p + 1, :], at[:, 1, lo:512],
                       start=st, stop=sp)

                deferred = []

                def issue_norm_a(p, it):
                    ys = ys_tiles[(p, it)]
                    rec = rec_pool.tile([64, 2, 512], F32, tag="rec")
                    nc.vector.reciprocal_approx_fast(
                        rec[:], ys[0:64, :, :])
                    deferred.append((p, it, rec))

                def issue_norm_b():
                    p, it, rec = deferred.pop(0)
                    ys = ys_tiles.pop((p, it))
                    nc.vector.tensor_mul(
                        yT[0:64, p, 512 * it:512 * (it + 1)],
                        ys[64:128, 0, :], rec[:, 0, :])
                    nc.vector.tensor_mul(
                        yT[64:128, p, 512 * it:512 * (it + 1)],
                        ys[64:128, 1, :], rec[:, 1, :])

                for g, (p, it, jt) in enumerate(steps):
                    if jt == 0:
                        ys_tiles[(p, it)] = ps_ys.tile(
                            [128, 2, 512], F32, tag="ys", name=f"ys{p}_{it}")
                    if it == 0 and jt == 0:
                        for m in dma_plan.get(p, ()):
                            wqk_dma(m, nc.gpsimd)
                        if p == 4:
                            # prefetch out-projection weights
                            nc.gpsimd.dma_start(
                                wo[:], w_oT.rearrange("a p b -> p a b"))
                    issue_sc(p, it, jt)
                    issue_exp(p, it, jt)
                    if g >= LAG:
                        pp, pit, pjt = steps[g - LAG]
                        issue_av(pp, pit, pjt)
                        if (pit, pjt) == (0, 3) or (pit, pjt) == (1, 7):
                            issue_norm_a(pp, pit)
                    if g >= LAG + 2:
                        pp, pit, pjt = steps[g - LAG - 2]
                        if (pit, pjt) == (0, 3) or (pit, pjt) == (1, 7):
                            issue_norm_b()
                    pb = flat_bursts[p]
                    for _ in range(sched.get(g, 0)):
                        if pb:
                            pb.pop(0)()
                for g in range(len(steps) - LAG, len(steps)):
                    pp, pit, pjt = steps[g]
                    issue_av(pp, pit, pjt)
                    if (pit, pjt) == (0, 3) or (pit, pjt) == (1, 7):
                        issue_norm_a(pp, pit)
                while deferred:
                    issue_norm_b()

            # ---------------- out^T projection ------------------------
            with (
                tc.tile_pool(name="ost", bufs=4) as out_pool,
                tc.tile_pool(name="ps3", bufs=4, space="PSUM") as ps3,
            ):
                for m2 in range(8):
                    pss = [ps3.tile([128, 512], F32, tag="ps3t",
                                    name=f"o{m2}_{i}") for i in range(2)]
                    for k in range(8):
                        for n in range(2):
                            mm(pss[n][:], wo[:, k, 128 * m2:128 * (m2 + 1)],
                               yT[:, k, 512 * n:512 * (n + 1)],
                               start=(k == 0), stop=(k == 7))
                    for n in range(2):
                        st = out_pool.tile([128, 512], F32)
                        nc.scalar.activation(st[:], pss[n][:], IDENT,
                                             bias=b_o_sb[:, m2:m2 + 1])
                        eng = nc.sync if n == 0 else nc.gpsimd
                        eng.dma_start(
                            outT[128 * m2:128 * (m2 + 1),
                                 512 * n:512 * (n + 1)], st[:])

    nc.compile()
    return nc


def _host_prep(x, w_qkv, b_qkv, w_out, b_out):
    bf = ml_dtypes.bfloat16
    x = np.asarray(x, dtype=np.float32)
    w_qkv = np.asarray(w_qkv, dtype=np.float32)
    b_qkv = np.asarray(b_qkv, dtype=np.float32)
    w_out = np.asarray(w_out, dtype=np.float32)
    b_out = np.asarray(b_out, dtype=np.float32)

    # [m, p, k, c] pre-tiled so each m-tile is one contiguous DMA
    w_qkT = np.ascontiguousarray(
        w_qkv[:2 * E].T.reshape(8, 128, 16, 128).transpose(2, 1, 0, 3)
    ).reshape(16, 128, 1024).astype(bf)
    b_qk = np.ascontiguousarray(
        b_qkv[:2 * E].reshape(16, 128).T).astype(np.float32)     # [128, 16]
    w_vT = np.concatenate(
        [w_qkv[2 * E:].T, b_qkv[2 * E:][None, :]], axis=0).astype(bf)
    # out^T projection stationary tiles:
    # w_oT[k][p][128m+c] = w_out[128m+c, 128k+p]
    w_oT = np.ascontiguousarray(w_out.T.reshape(8, 128, 1024)).astype(bf)
    b_o = np.ascontiguousarray(b_out.reshape(8, 128).T).astype(np.float32)

    j = np.arange(128)[:, None]
    i = np.arange(128)[None, :]
    tri1 = (j <= i).astype(np.float32)
    tri = np.concatenate([tri1, tri1], axis=1).astype(bf)        # [128, 256]

    ones = np.ones((1, T), dtype=np.float32)
    per_core = []
    for cc in range(N_CORES):
        xTc = np.concatenate([x[cc].T, ones], axis=0).astype(bf)
        per_core.append({
            "xT": xTc, "w_qkT": w_qkT, "b_qk": b_qk, "w_vT": w_vT,
            "w_oT": w_oT, "b_o": b_o, "tri": tri,
        })
    return per_core


def kernel(x, w_qkv, b_qkv, w_out, b_out, cos_tab, sin_tab):
    # cos_tab/sin_tab unused: the module applies the identical rotation R to
    # q and k at every position and R R^T = I cancels inside q @ k^T.
    if "nc" not in _cache:
        _cache["nc"] = _build()
    nc = _cache["nc"]
    in_maps = _host_prep(x, w_qkv, b_qkv, w_out, b_out)
    res = run_bass_kernel_spmd(nc, in_maps, list(range(N_CORES)),
                               trace=False)
    out = np.stack([res.results[c]["outT"].T for c in range(N_CORES)], axis=0)
    return np.ascontiguousarray(out).astype(np.float32)


def run_traced(x, w_qkv, b_qkv, w_out, b_out, cos_tab, sin_tab):
    """Like kernel() but with NTFF profiling; returns (out, exec_time_ns,
    trace_path)."""
    if "nc" not in _cache:
        _cache["nc"] = _build()
    nc = _cache["nc"]
    in_maps = _host_prep(x, w_qkv, b_qkv, w_out, b_out)
    res = run_bass_kernel_spmd(nc, in_maps, list(range(N_CORES)), trace=True)
    out = np.stack([res.results[c]["outT"].T for c in range(N_CORES)], axis=0)
    out = np.ascontiguousarray(out).astype(np.float32)
    trace_path = None
    if res.instructions_and_trace is not None:
        trace_path = res.instructions_and_trace[1]
    return out, res.exec_time_ns, trace_path
